# revision 1
# baseline (speedup 1.0000x reference)
"""Trainium2 Bass kernel for nn_CFC_Reformer (CFC + Reformer attention block).

Contract: kernel(**inputs) takes the FULL inputs (x: [8,256,96,96] f32 plus
small conv/attention params), shards x along batch across 8 NeuronCores
(pure data parallel, params replicated), runs one fused Bass/Tile program
per core, and gathers the full [8,128,96,96] f32 output.

Per-core pipeline (one image [256,96,96]):
  A. xr = SiLU(BN(conv3x3(x, w_red)))       -- 18 fp32r matmuls/tile in PSUM
  B. PSP pooling -> 50 token sums; Reformer bucket argmax/argsort built from
     compare ops + one-hot gather matmuls; kwT = (k_s @ w_q)^T
  C. s = kwT^T @ xr ; att = softmax_k(s) ; ctx = v_s^T @ att ; y = SiLU(BN(w_la1 @ ctx))
  D. mask = tanh(conv3x3(y, w_la2)) ; out = xr + ctx * (1 + mask)

Spatial layout on chip: width padded 96 -> 128 (zero pad col 0, cols 97..127)
so a 512-column tile is exactly 4 image rows and 3x3 conv taps are pure
column offsets.
"""

import numpy as np

import concourse.bass as bass
import concourse.bacc as bacc
import concourse.mybir as mybir
import concourse.tile as tile
from concourse.bass_utils import run_bass_kernel_spmd

F32 = mybir.dt.float32
F32R = mybir.dt.float32r
AF = mybir.ActivationFunctionType
ALU = mybir.AluOpType
AX = mybir.AxisListType

# Problem shapes (hardcoded per the harness contract).
B, H, W = 8, 96, 96
CIN, COUT, QD, NH = 256, 128, 32, 8
LA_MID = 16
EPS = 1e-5
WP = 98                     # padded row width (1 zero col each side)
NPIX = H * WP               # 12288 padded pixels
TCOLS = 4 * WP              # psum tile = 4 image rows
NT = NPIX // TCOLS          # 24 spatial tiles
RPB = 8                     # conv-A rows per block
NBLK = H // RPB             # 12 conv blocks
XIN_SZ = 1 + 10 * WP + 8    # input block tile: guard col + 10 halo rows + slack
XTOT = 1 + 98 * WP + 8      # host-padded x: guard col + 98 padded rows + slack
Y_G = WP + 1                # y top guard (one padded row + guard col)
Y_SZ = Y_G + NPIX + Y_G + 6 # y tile with top/bottom zero guards
NTOK = 50                   # 36 + 9 + 4 + 1 PSP tokens
BIGF = 1.0e9

_BUILD_CACHE = {}
F32R_INPUTS = frozenset({"w1t", "wla1T", "wla2t", "ones8", "ones18"})


def _r32(ap):
    return ap.bitcast(F32R)


def _expand_vec3(v):
    out = np.zeros((96, 1), np.float32)
    for sft in range(3):
        out[sft * 32:sft * 32 + LA_MID, 0] = v
    return out


def _expand_la1(w_la1):
    out = np.zeros((128, 96), np.float32)
    for sft in range(3):
        out[:, sft * 32:sft * 32 + LA_MID] = w_la1.T
    return out


def _round_fp32r(a):
    """Round-to-nearest-even to fp32r (e8m13) so PE truncation is exact."""
    u = np.ascontiguousarray(a, np.float32).view(np.uint32)
    r = (u + 0x1FF + ((u >> 10) & 1)) & np.uint32(0xFFFFFC00)
    return r.view(np.float32)


def _host_prep(inp):
    """Fold BN into conv weights and lay every parameter out exactly as the
    SBUF tiles expect ([partition, free], contraction on partitions)."""
    f = np.float32
    w_red = np.asarray(inp["w_red"], f)
    binv = np.asarray(inp["bng"], f) / np.sqrt(np.asarray(inp["bnv"], f) + EPS)
    bnbias = np.asarray(inp["bnb"], f) - np.asarray(inp["bnm"], f) * binv
    wf = w_red * binv[:, None, None, None]          # [COUT, CIN, 3, 3]

    w1t = np.empty((128, 2304), f)
    for kc in range(2):
        for dy in range(3):
            for dx in range(3):
                t = kc * 9 + dy * 3 + dx
                # [ci_local, co]
                w1t[:, t * 128:(t + 1) * 128] = wf[:, kc * 128:(kc + 1) * 128, dy, dx].T

    w_la2 = np.asarray(inp["w_la2"], f)             # [COUT, LA_MID, 3, 3]
    # K-packed: partitions (dx-shift s)*32 + ci (32-stride for legal engine
    # partition starts; odd half zero), one matmul per dy tap
    wla2t = np.zeros((96, 3 * 128), f)
    for dy in range(3):
        for sft in range(3):
            wla2t[sft * 32:sft * 32 + LA_MID, dy * 128:(dy + 1) * 128] = \
                w_la2[:, :, dy, sft].T

    lasc = np.asarray(inp["lag"], f) / np.sqrt(np.asarray(inp["lav"], f) + EPS)
    labi = np.asarray(inp["lab"], f) - np.asarray(inp["lam"], f) * lasc

    inv_area = np.concatenate([
        np.full(36, 1.0 / 256), np.full(9, 1.0 / 1024),
        np.full(4, 1.0 / 2304), np.full(1, 1.0 / 9216)]).astype(f)

    jlt = np.tril(np.ones((NH, NH), f), k=-1)        # jlt[i,j] = 1 if j < i
    p_iota = np.tile(np.arange(NH, dtype=f), (NH, 1))

    lsh = np.asarray(inp["lsh"], f)
    b_k = np.asarray(inp["b_k"], f)

    return {
        "w1t": _round_fp32r(w1t),
        "bnbias": np.ascontiguousarray(bnbias.reshape(128, 1)),
        "wkT": np.ascontiguousarray(np.asarray(inp["w_k"], f).T),      # [128,32]
        "wvT": np.ascontiguousarray(np.asarray(inp["w_v"], f).T),      # [128,128]
        "wq": np.ascontiguousarray(np.asarray(inp["w_q"], f)),         # [32,128]
        "bq": np.ascontiguousarray(np.asarray(inp["b_q"], f).reshape(QD, 1)),
        "lshT": np.ascontiguousarray(lsh.T),                           # [32,8]
        "lshbk": np.ascontiguousarray((lsh @ b_k).reshape(NH, 1)),
        "bk8": np.ascontiguousarray(np.tile(b_k, (NH, 1))),            # [8,32]
        "bv8": np.ascontiguousarray(np.tile(np.asarray(inp["b_v"], f), (NH, 1))),
        "wla1T": _round_fp32r(_expand_la1(np.asarray(inp["w_la1"], f))),  # [128,96]
        "lasc": np.ascontiguousarray(_expand_vec3(lasc)),
        "labi": np.ascontiguousarray(_expand_vec3(labi)),
        "wla2t": _round_fp32r(wla2t),
        "invarea": np.ascontiguousarray(np.tile(inv_area, (NH, 1))),   # [8,50]
        "iota50": np.ascontiguousarray(
            np.tile(np.arange(NTOK, dtype=f), (NH, 1))),               # [8,50]
        "jlt": jlt,
        "piota": p_iota,
        "ones8": np.ones((NH, 1), f),
        "ones18": np.ones((1, NH), f),
        "ident": np.eye(128, dtype=f),
        "yzero": np.zeros((96, Y_SZ), f),
    }


def build_program():
    """Build the single-core SPMD Bass/Tile program. Same program runs on all
    8 cores; only the 'x' input differs per core."""
    nc = bacc.Bacc("TRN2", target_bir_lowering=False, debug=False)

    di = {}
    def din(name, shape):
        di[name] = nc.dram_tensor(name, list(shape), F32, kind="ExternalInput").ap()

    din("x", (CIN, XTOT))
    for name, shape in [
        ("w1t", (128, 2304)), ("bnbias", (128, 1)), ("wkT", (128, QD)),
        ("wvT", (128, 128)), ("wq", (QD, 128)), ("bq", (QD, 1)),
        ("lshT", (QD, NH)), ("lshbk", (NH, 1)), ("bk8", (NH, QD)),
        ("bv8", (NH, 128)), ("wla1T", (128, 96)), ("lasc", (96, 1)),
        ("labi", (96, 1)), ("wla2t", (96, 3 * 128)),
        ("invarea", (NH, NTOK)), ("iota50", (NH, NTOK)), ("jlt", (NH, NH)),
        ("piota", (NH, NH)), ("ones8", (NH, 1)), ("ones18", (1, NH)),
        ("ident", (128, 128)), ("yzero", (96, Y_SZ)),
    ]:
        din(name, shape)
    out_d = nc.dram_tensor("out", [COUT, NPIX], F32, kind="ExternalOutput").ap()

    with tile.TileContext(nc) as tc:
      # one long-lived pool holds every persistent tile (unique tag = own slot)
      with tc.tile_pool(name="perm", bufs=1) as perm:
        def ptile(name, shape, dt=F32):
            return perm.tile(list(shape), dt, name=name, tag=name)

        xin = [[ptile(f"xin{s}{kc}", [128, XIN_SZ], F32R) for kc in range(2)]
               for s in range(2)]
        xr = ptile("xr", [128, NPIX], F32R)
        ctx_s = ptile("ctx_s", [128, NPIX], F32R)
        y_s = ptile("y_s", [96, Y_SZ], F32R)
        rowsum6 = ptile("rowsum6", [128, 576])
        S_s = ptile("S_s", [128, 64])

        sb = {}
        for name, shape in [
            ("w1t", (128, 2304)), ("bnbias", (128, 1)), ("wkT", (128, QD)),
            ("wvT", (128, 128)), ("wq", (QD, 128)), ("bq", (QD, 1)),
            ("lshT", (QD, NH)), ("lshbk", (NH, 1)), ("bk8", (NH, QD)),
            ("bv8", (NH, 128)), ("wla1T", (128, 96)), ("lasc", (96, 1)),
            ("labi", (96, 1)), ("wla2t", (96, 3 * 128)),
            ("invarea", (NH, NTOK)), ("iota50", (NH, NTOK)), ("jlt", (NH, NH)),
            ("piota", (NH, NH)), ("ones8", (NH, 1)), ("ones18", (1, NH)),
            ("ident", (128, 128)),
        ]:
            dt_ = F32R if name in F32R_INPUTS else F32
            sb[name] = ptile("sb_" + name, list(shape), dt_)
            src = di[name][:, :].bitcast(dt_) if dt_ is F32R else di[name][:, :]
            nc.sync.dma_start(out=sb[name][:, :], in_=src)

        # y guards/pads must be zero for the 3x3 mask conv (DMA'd zeros keep
        # the tile fp32r end-to-end; memset can't encode an fp32r value type)
        nc.sync.dma_start(out=y_s[:, :], in_=di["yzero"][:, :].bitcast(F32R))

        # ================= Phase A: conv3x3 + BN + SiLU -> xr =============
        with tc.tile_pool(name="apsum", bufs=4, space="PSUM") as apool:
            for b in range(NBLK):
                y0 = RPB * b
                sel = b % 2
                for kc in range(2):
                    # host-padded x frames the image with zeros, so every
                    # block is one full-tile contiguous window (incl guard)
                    nc.sync.dma_start(
                        out=xin[sel][kc][:, :],
                        in_=di["x"][kc * 128:(kc + 1) * 128,
                                    y0 * WP:y0 * WP + XIN_SZ].bitcast(F32R))
                for r0 in (0, 4):
                    ps = apool.tile([128, TCOLS], F32, tag="apsum")
                    first = True
                    for kc in range(2):
                        for dy in range(3):
                            for dx in range(3):
                                t = kc * 9 + dy * 3 + dx
                                off = 1 + (r0 + dy) * WP + dx - 1
                                nc.tensor.matmul(
                                    ps[:, :],
                                    sb["w1t"][:, t * 128:(t + 1) * 128],
                                    xin[sel][kc][:, off:off + TCOLS],
                                    start=first, stop=(t == 17))
                                first = False
                    col0 = (y0 + r0) * WP
                    nc.scalar.activation(
                        xr[:, col0:col0 + TCOLS], ps[:, :], AF.Silu,
                        bias=sb["bnbias"][:, 0:1])
                # PSP stage 1: 16-wide row sums for this block's 8 rows
                xrb = xr[:, y0 * WP:(y0 + RPB) * WP].rearrange(
                    "p (y c) -> p y c", c=WP)[:, :, 1:1 + W].rearrange(
                    "p y (j u) -> p y j u", u=16)
                nc.vector.tensor_reduce(
                    rowsum6[:, b * 48:(b + 1) * 48].rearrange(
                        "p (y j) -> p y j", j=6),
                    xrb, axis=AX.X, op=ALU.add)

        # ================= Phase B: tokens + reformer gather ==============
        # S6 [128,36]: column sums of rowsum6 over 16-row groups
        nc.vector.tensor_reduce(
            S_s[:, 0:36].rearrange("p (i j) -> p i j", j=6),
            rowsum6[:, :].rearrange("p (i u j) -> p i j u", i=6, u=16, j=6),
            axis=AX.X, op=ALU.add)
        s3t = ptile("s3t", [128, 18])
        nc.vector.tensor_reduce(
            s3t[:, :].rearrange("p (i a j) -> p i a j", i=3, a=2, j=3),
            S_s[:, 0:36].rearrange("p (i a j b) -> p i a j b", i=3, a=2, j=3, b=2),
            axis=AX.X, op=ALU.add)
        nc.vector.tensor_reduce(
            S_s[:, 36:45].rearrange("p (i j) -> p i j", j=3),
            s3t[:, :].rearrange("p (i a j) -> p i j a", i=3, a=2, j=3),
            axis=AX.X, op=ALU.add)
        s2t = ptile("s2t", [128, 12])
        nc.vector.tensor_reduce(
            s2t[:, :].rearrange("p (i a j) -> p i a j", i=2, a=3, j=2),
            S_s[:, 0:36].rearrange("p (i a j b) -> p i a j b", i=2, a=3, j=2, b=3),
            axis=AX.X, op=ALU.add)
        nc.vector.tensor_reduce(
            S_s[:, 45:49].rearrange("p (i j) -> p i j", j=2),
            s2t[:, :].rearrange("p (i a j) -> p i j a", i=2, a=3, j=2),
            axis=AX.X, op=ALU.add)
        nc.vector.tensor_reduce(
            S_s[:, 49:50], S_s[:, 0:36].rearrange("p (i j) -> p i j", j=6),
            axis=AX.XY, op=ALU.add)

        kS = ptile("kS", [QD, NTOK])
        vS = ptile("vS", [128, NTOK])
        ktok8 = ptile("ktok8", [NH, QD])
        vtok8 = ptile("vtok8", [NH, 128])
        Lsc = ptile("Lsc", [NH, NTOK])
        eqt = ptile("eqt", [NH, NTOK])
        t1 = ptile("t1", [NH, NTOK])
        t2 = ptile("t2", [NH, NTOK])
        maxv = ptile("maxv", [NH, 1])
        bmin = ptile("bmin", [NH, 1])
        bT = ptile("bT", [1, NH], F32R)
        ranksrc = ptile("ranksrc", [NH, NH])
        eqm = ptile("eqm", [NH, NH])
        rank = ptile("rank", [NH, 1])
        onehot = ptile("onehot", [NH, NH])
        ksT = ptile("ksT", [QD, NH])
        vs_g = ptile("vs_g", [NH, 128], F32R)
        kwT = ptile("kwT", [128, NH], F32R)
        sbias = ptile("sbias", [NH, 1])

        with tc.tile_pool(name="bpsum", bufs=2, space="PSUM") as bpool:
            kS_p = bpool.tile([QD, NTOK], F32, tag="b1")
            nc.tensor.matmul(kS_p[:, :], sb["wkT"][:, :], S_s[:, 0:NTOK],
                             start=True, stop=True)
            nc.scalar.copy(kS[:, :], kS_p[:, :])
            vS_p = bpool.tile([128, NTOK], F32, tag="b2")
            nc.tensor.matmul(vS_p[:, :], sb["wvT"][:, :], S_s[:, 0:NTOK],
                             start=True, stop=True)
            nc.scalar.copy(vS[:, :], vS_p[:, :])

            # bucket logits over all 50 tokens (area-normalized + lsh@b_k)
            L_p = bpool.tile([NH, NTOK], F32, tag="b1")
            nc.tensor.matmul(L_p[:, :], sb["lshT"][:, :], kS[:, :],
                             start=True, stop=True)
            nc.vector.tensor_tensor(Lsc[:, :], L_p[:, :], sb["invarea"][:, :],
                                    op=ALU.mult)
            nc.vector.tensor_scalar_add(Lsc[:, :], Lsc[:, :], sb["lshbk"][:, 0:1])
            # argmax over tokens (first occurrence)
            nc.vector.tensor_reduce(maxv[:, :], Lsc[:, :], axis=AX.X, op=ALU.max)
            nc.vector.tensor_scalar(eqt[:, :], Lsc[:, :], maxv[:, 0:1], None,
                                    op0=ALU.is_equal)
            nc.vector.tensor_tensor(t1[:, :], eqt[:, :], sb["iota50"][:, :],
                                    op=ALU.mult)
            nc.vector.tensor_scalar(t2[:, :], eqt[:, :], -BIGF, BIGF,
                                    op0=ALU.mult, op1=ALU.add)
            nc.vector.tensor_tensor(t1[:, :], t1[:, :], t2[:, :], op=ALU.add)
            nc.vector.tensor_reduce(bmin[:, :], t1[:, :], axis=AX.X, op=ALU.min)

            # stable argsort rank of the 8 bucket ids
            bT_p = bpool.tile([1, NH], F32, tag="b1")
            nc.tensor.matmul(bT_p[:, :], bmin[:, :], sb["ident"][0:NH, 0:NH],
                             start=True, stop=True)
            nc.scalar.copy(bT[:, :], bT_p[:, :])
            Bij_p = bpool.tile([NH, NH], F32, tag="b2")
            nc.tensor.matmul(Bij_p[:, :], sb["ones18"][:, :], bT[:, :],
                             start=True, stop=True)
            nc.vector.tensor_scalar(ranksrc[:, :], Bij_p[:, :], bmin[:, 0:1], None,
                                    op0=ALU.is_lt)
            nc.vector.tensor_scalar(eqm[:, :], Bij_p[:, :], bmin[:, 0:1], None,
                                    op0=ALU.is_equal)
            nc.vector.tensor_tensor(eqm[:, :], eqm[:, :], sb["jlt"][:, :],
                                    op=ALU.mult)
            nc.vector.tensor_tensor(ranksrc[:, :], ranksrc[:, :], eqm[:, :],
                                    op=ALU.add)
            nc.vector.tensor_reduce(rank[:, :], ranksrc[:, :], axis=AX.X, op=ALU.add)
            nc.vector.tensor_scalar(onehot[:, :], sb["piota"][:, :], rank[:, 0:1],
                                    None, op0=ALU.is_equal)

            # first-8 tokens to [token, feat] layout (+ mean scale & bias)
            kt_p = bpool.tile([NH, QD], F32, tag="b1")
            nc.tensor.transpose(kt_p[:, :], kS[:, 0:NH], sb["ident"][0:QD, 0:QD])
            nc.vector.tensor_scalar(ktok8[:, :], kt_p[:, :], 1.0 / 256, None,
                                    op0=ALU.mult)
            nc.vector.tensor_tensor(ktok8[:, :], ktok8[:, :], sb["bk8"][:, :],
                                    op=ALU.add)
            vt_p = bpool.tile([NH, 128], F32, tag="b2")
            nc.tensor.transpose(vt_p[:, :], vS[:, 0:NH], sb["ident"][:, :])
            nc.vector.tensor_scalar(vtok8[:, :], vt_p[:, :], 1.0 / 256, None,
                                    op0=ALU.mult)
            nc.vector.tensor_tensor(vtok8[:, :], vtok8[:, :], sb["bv8"][:, :],
                                    op=ALU.add)

            # gather sorted tokens, fold w_q, score bias
            ksT_p = bpool.tile([QD, NH], F32, tag="b1")
            nc.tensor.matmul(ksT_p[:, :], ktok8[:, :], onehot[:, :],
                             start=True, stop=True)
            nc.scalar.copy(ksT[:, :], ksT_p[:, :])
            vs_p = bpool.tile([NH, 128], F32, tag="b2")
            nc.tensor.matmul(vs_p[:, :], onehot[:, :], vtok8[:, :],
                             start=True, stop=True)
            nc.scalar.copy(vs_g[:, :], vs_p[:, :])
            kw_p = bpool.tile([128, NH], F32, tag="b1")
            nc.tensor.matmul(kw_p[:, :], sb["wq"][:, :], ksT[:, :],
                             start=True, stop=True)
            nc.scalar.copy(kwT[:, :], kw_p[:, :])
            sb_p = bpool.tile([NH, 1], F32, tag="b2")
            nc.tensor.matmul(sb_p[:, :], ksT[:, :], sb["bq"][:, :],
                             start=True, stop=True)
            nc.scalar.copy(sbias[:, :], sb_p[:, :])

        # ============ Phase C1: attention (ACT runs exp only) =============
        with (
            tc.tile_pool(name="cpool", bufs=2) as cpool,
            tc.tile_pool(name="cps_s", bufs=2, space="PSUM") as ps_s,
            tc.tile_pool(name="cps_z1", bufs=1, space="PSUM") as ps_z1,
            tc.tile_pool(name="cps_z2", bufs=1, space="PSUM") as ps_z2,
            tc.tile_pool(name="cps_ctx", bufs=2, space="PSUM") as ps_ctx,
        ):
            for t in range(NT):
                c0 = t * TCOLS
                s_p = ps_s.tile([NH, TCOLS], F32, tag="s")
                nc.tensor.matmul(s_p[:, :], kwT[:, :], xr[:, c0:c0 + TCOLS],
                                 start=True, stop=True)
                e_t = cpool.tile([NH, TCOLS], F32R, tag="e")
                nc.scalar.activation(e_t[:, :], s_p[:, :], AF.Exp,
                                     bias=sbias[:, 0:1])
                z_p = ps_z1.tile([1, TCOLS], F32, tag="z")
                nc.tensor.matmul(z_p[:, :], sb["ones8"][:, :], e_t[:, :],
                                 start=True, stop=True)
                rc_t = cpool.tile([1, TCOLS], F32R, tag="rc")
                with nc.allow_low_precision(reason="fp32r feed to PE broadcast"):
                    nc.vector.reciprocal(rc_t[:, :], z_p[:, :])
                zr_p = ps_z2.tile([NH, TCOLS], F32, tag="zr")
                nc.tensor.matmul(zr_p[:, :], sb["ones18"][:, :], rc_t[:, :],
                                 start=True, stop=True)
                nc.vector.tensor_tensor(e_t[:, :], e_t[:, :], zr_p[:, :],
                                        op=ALU.mult)
                ctx_p = ps_ctx.tile([128, TCOLS], F32, tag="ctx")
                nc.tensor.matmul(ctx_p[:, :], vs_g[:, :], e_t[:, :],
                                 start=True, stop=True)
                nc.scalar.copy(ctx_s[:, c0:c0 + TCOLS], ctx_p[:, :])

        # keep exp and silu in separate contiguous ACT runs (LUT reloads)
        tc.no_sync_barrier()

        # ===== Phase C2 (silu -> packed y3) + Phase D (tanh shares LUT) ====
        with (
            tc.tile_pool(name="dpool", bufs=3) as dpool,
            tc.tile_pool(name="cps_yp", bufs=2, space="PSUM") as ps_yp,
            tc.tile_pool(name="dpsum", bufs=2, space="PSUM") as dps,
        ):
            for t in range(NT):
                c0 = t * TCOLS
                yp_p = ps_yp.tile([96, TCOLS], F32, tag="yp")
                nc.tensor.matmul(yp_p[:, :], sb["wla1T"][:, :],
                                 ctx_s[:, c0:c0 + TCOLS], start=True, stop=True)
                for sft in range(3):
                    pslc = yp_p[sft * 32:sft * 32 + LA_MID, :]
                    pv = pslc.rearrange("p (r c) -> p r c", c=WP)[:, :, 1:1 + W]
                    dst = y_s[sft * 32:sft * 32 + LA_MID,
                              Y_G + c0 - (sft - 1):Y_G + c0 - (sft - 1) + TCOLS
                              ].rearrange("p (r c) -> p r c", c=WP)[:, :, 1:1 + W]
                    nc.scalar.activation(
                        dst, pv, AF.Silu,
                        bias=sb["labi"][sft * 32:sft * 32 + LA_MID, 0:1],
                        scale=sb["lasc"][sft * 32:sft * 32 + LA_MID, 0:1])

            for t in range(NT):
                c0 = t * TCOLS
                m_p = dps.tile([128, TCOLS], F32, tag="m")
                for dy in range(3):
                    off = Y_G + c0 + (dy - 1) * WP
                    nc.tensor.matmul(
                        m_p[:, :],
                        sb["wla2t"][:, dy * 128:(dy + 1) * 128],
                        y_s[:, off:off + TCOLS],
                        start=(dy == 0), stop=(dy == 2))
                msk = dpool.tile([128, TCOLS], F32, tag="msk")
                nc.scalar.activation(msk[:, :], m_p[:, :], AF.Tanh)
                o_t = dpool.tile([128, TCOLS], F32, tag="o")
                nc.vector.tensor_tensor(o_t[:, :], ctx_s[:, c0:c0 + TCOLS],
                                        msk[:, :], op=ALU.mult)
                nc.vector.tensor_tensor(o_t[:, :], o_t[:, :],
                                        ctx_s[:, c0:c0 + TCOLS], op=ALU.add)
                nc.vector.tensor_tensor(o_t[:, :], o_t[:, :],
                                        xr[:, c0:c0 + TCOLS], op=ALU.add)
                nc.sync.dma_start(out=out_d[:, c0:c0 + TCOLS], in_=o_t[:, :])

    nc.compile()
    return nc


def get_program():
    if "nc" not in _BUILD_CACHE:
        _BUILD_CACHE["nc"] = build_program()
    return _BUILD_CACHE["nc"]


def pad_x(xb):
    """[256,96,96] -> zero-framed [256, XTOT] fp32r-rounded."""
    xp = np.zeros((CIN, XTOT), np.float32)
    body = np.zeros((CIN, 98, WP), np.float32)
    body[:, 1:1 + H, 1:1 + W] = xb
    xp[:, 1:1 + 98 * WP] = body.reshape(CIN, 98 * WP)
    return _round_fp32r(xp)


def unpad_out(flat):
    """[128, 12288] padded rows -> [128, 96, 96]."""
    return np.ascontiguousarray(flat.reshape(COUT, H, WP)[:, :, 1:1 + W])


def kernel(**inputs):
    x = np.ascontiguousarray(np.asarray(inputs["x"], np.float32))
    assert x.shape == (B, CIN, H, W)
    weights = _host_prep(inputs)
    nc = get_program()
    in_maps = [dict(weights, x=pad_x(x[b])) for b in range(B)]
    res = run_bass_kernel_spmd(nc, in_maps, list(range(B)))
    out = np.stack([unpad_out(res.results[b]["out"]) for b in range(B)], axis=0)
    return out.astype(np.float32)

